# revision 8
# baseline (speedup 1.0000x reference)
"""Trainium2 Bass kernel for nn_Clar_Loss (NSML fusion-clarity MSE loss).

reference:
    x = (t+1)/2 ; s = sml(x) (8-neighbor abs-diff stencil, edge-replicate)
    nsml = G (*) s^2  (3x3 gaussian [[1,2,1],[2,4,2],[1,2,1]]/16, zero pad)
    loss = mean((nsml(A) - nsml(B))^2)

Algebra used here:
    sml((t+1)/2) = sml(t)/2          (translation invariant, pos. homogeneous)
    loss = sum((Graw (*) q)^2) / (N * 4096),  q = sA^2 - sB^2 (raw sml)
with Graw = [[1,2,1],[2,4,2],[1,2,1]] (integer), separable [1,2,1](x)[1,2,1].

Sharding: pure data-parallel over the batch dim (16 -> 2 per core); each core
returns a partial sum of (Graw(*)q)^2; host sums and rescales.

Layout: inputs are pre-padded on host to [H, W+2] (edge-replicated columns),
so every row-tile is ONE full [128, 1026] fp16 cast-DMA (partition p <->
image row off+p; off=0 / r0-2 / 896). All vertical/partition shifts are
folded into host-built 128x128 fp16 stationary matrices; image-boundary
clamping and the zero padding of s^2 are encoded as matrix-column edits, so
no partition ever needs pad data. The 4 |diff| fields are produced by a
custom fused DVE absdiff op, keeping each matmul's sync-wait count at 1.
"""

import os
from contextlib import ExitStack

import numpy as np

B, C, H, W = 16, 3, 1024, 1024
NCORES = 8
BPC = B // NCORES            # batch per core
NSITE = BPC * C              # image pairs per core
DIAG_W = 0.707
TILE_R = 124
NT = (H + TILE_R - 1) // TILE_R          # 9
FINAL_SCALE = 1.0 / (float(B * C * H * W) * 4096.0)

_CACHE = {}


def _tile_geom(t):
    """(r0, R, off): output rows [r0, r0+R), partition p <-> image row off+p."""
    r0 = t * TILE_R
    R = min(TILE_R, H - r0)
    if t == 0:
        off = 0
    elif R < TILE_R or r0 + 126 > H:
        off = H - 128
    else:
        off = r0 - 2
    return r0, R, off


# --------------------------------------------------------------------------
# host-built stationary matrices (lhsT layout [k, m]: out[m] += M[k,m]*in[k])
# --------------------------------------------------------------------------

def _matrices_for_tile(t):
    r0, R, off = _tile_geom(t)

    def sml_valid(m):
        r = off + m
        if not (max(r0 - 1, 0) <= r <= min(r0 + R, H - 1)):
            return False
        if m > 126 and r != H - 1:
            return False
        if m < 1 and r != 0:
            return False
        return True

    dv = np.zeros((128, 128), np.float32)    # dv[m] = x[m] - x[m-1]
    for m in range(1, 128):
        dv[m, m] = 1.0
        dv[m - 1, m] = -1.0

    av = np.zeros((128, 128), np.float32)    # a_v terms
    ihf = np.zeros((128, 128), np.float32)   # a_h at f
    ihf1 = np.zeros((128, 128), np.float32)  # a_h at f+1
    i71 = np.zeros((128, 128), np.float32)   # a_1 at f
    s71 = np.zeros((128, 128), np.float32)   # a_1[m+1] at f+1
    i72 = np.zeros((128, 128), np.float32)   # a_2 at f (img f)
    s72 = np.zeros((128, 128), np.float32)   # a_2[m+1] at f-1

    for m in range(128):
        if not sml_valid(m):
            continue
        r = off + m
        ihf[m, m] += 1.0           # |c-lf|
        ihf1[m, m] += 1.0          # |c-rt|
        if r >= 1:
            av[m, m] += 1.0        # |c-up| = a_v[m]
        if r <= H - 2:
            av[m + 1, m] += 1.0    # |c-dn| = a_v[m+1]
        if r == 0:                 # up-row clamps to own row
            ihf[m, m] += DIAG_W    # |c-ul| -> a_h[m, f]
            ihf1[m, m] += DIAG_W   # |c-ur| -> a_h[m, f+1]
        else:
            i71[m, m] += DIAG_W    # |c-ul| = a_1[m]
            i72[m, m] += DIAG_W    # |c-ur| = a_2[m]
        if r == H - 1:             # down-row clamps to own row
            ihf1[m, m] += DIAG_W   # |c-dr| -> a_h[m, f+1]
            ihf[m, m] += DIAG_W    # |c-dl| -> a_h[m, f]
        else:
            s71[m + 1, m] += DIAG_W  # |c-dr| = a_1[m+1] at f+1
            s72[m + 1, m] += DIAG_W  # |c-dl| = a_2[m+1] at f-1

    tri = np.zeros((128, 128), np.float32)
    for m in range(128):
        r = off + m
        if not (r0 <= r <= r0 + R - 1):
            continue
        for dr in (-1, 0, 1):
            k = m + dr
            if 0 <= k <= 127 and 0 <= off + k <= H - 1:
                tri[k, m] = 2.0 if dr == 0 else 1.0

    mats = {"DV": dv, "AV": av, "IHF": ihf, "IHF1": ihf1, "I71": i71,
            "S71": s71, "I72": i72, "S72": s72, "TRI": tri}
    return {k: v.astype(np.float16) for k, v in mats.items()}


def _build_weights():
    slots = {}
    packed = []
    index = {}
    for t in range(NT):
        for name, mat in _matrices_for_tile(t).items():
            key = mat.tobytes()
            if key not in slots:
                slots[key] = len(packed)
                packed.append(mat)
            index[(t, name)] = slots[key]
    ones = np.zeros((128, 128), np.float16)
    ones[:, 0] = 1.0
    index[("ones",)] = len(packed)
    packed.append(ones)
    wts = np.concatenate(packed, axis=1)  # [128, NW*128]
    return np.ascontiguousarray(wts), index


# --------------------------------------------------------------------------
# custom DVE op: absdiff  out = |in0 - in1|
# --------------------------------------------------------------------------

def _register_absdiff():
    from concourse import dve_ops
    from concourse.dve_spec import Spec, Src0, Src1, maxx, lower
    from concourse.dve_uop import DveOpSpec

    if any(op.name == "ABSDIFF_ANT" for op in dve_ops.OPS):
        return next(op for op in dve_ops.OPS if op.name == "ABSDIFF_ANT")

    spec = Spec(
        body=maxx(Src0 - Src1, Src1 - Src0),
        reference=lambda in0, in1, s0, s1, imm2: np.abs(
            in0.astype(np.float32) - in1.astype(np.float32)
        ),
    )
    opcode = max(dve_ops._SUB_OPCODE_FOR_NAME.values()) + 1
    assert opcode < 0x20
    shas = {}
    for ver in ("v3", "v4"):
        s = DveOpSpec(
            name="ABSDIFF_ANT",
            opcode=opcode,
            uops=lower(spec, ver=ver),
            rd1_en=True,
        )
        shas[ver] = s.sha(ver)
    op = dve_ops.DveOp("ABSDIFF_ANT", spec, subdim=False, uops_sha=shas)
    dve_ops.OPS.append(op)
    dve_ops._SUB_OPCODE_FOR_NAME["ABSDIFF_ANT"] = opcode
    dve_ops.CUSTOM_DVE_SPECS["ABSDIFF_ANT"] = spec
    return op


# --------------------------------------------------------------------------
# kernel build
# --------------------------------------------------------------------------

def _build(wts_np):
    import concourse.bass as bass
    import concourse.tile as tile
    from concourse import bacc, mybir

    F16 = mybir.dt.float16
    F32 = mybir.dt.float32
    AF = mybir.ActivationFunctionType
    OP = mybir.AluOpType

    absdiff = _register_absdiff()
    native_abs = bool(int(os.environ.get("CLAR_NATIVE_ABS", "1")))

    nc = bacc.Bacc()
    dA = nc.dram_tensor("TA", [NSITE, H, W + 2], F32, kind="ExternalInput")
    dB = nc.dram_tensor("TB", [NSITE, H, W + 2], F32, kind="ExternalInput")
    dW = nc.dram_tensor("WTS", list(wts_np.shape), F16, kind="ExternalInput")
    dO = nc.dram_tensor("OUT", [1, 1], F32, kind="ExternalOutput")

    with tile.TileContext(nc) as tc, ExitStack() as ctx:
        persist = ctx.enter_context(tc.tile_pool(name="persist", bufs=1))
        # bufs=4 with 4 DMAs/site-tile: slot reuse lands on the SAME
        # round-robin DMA lane, so the reload WAW is same-proc
        xp = ctx.enter_context(tc.tile_pool(name="xp", bufs=4))
        fields = ctx.enter_context(tc.tile_pool(name="fields", bufs=2))
        tails = ctx.enter_context(tc.tile_pool(name="tails", bufs=2))
        accs = ctx.enter_context(tc.tile_pool(name="accs", bufs=8))
        ps_sml = ctx.enter_context(tc.tile_pool(name="ps_sml", bufs=1, space="PSUM"))
        ps_dv = ctx.enter_context(tc.tile_pool(name="ps_dv", bufs=1, space="PSUM"))
        ps_r = ctx.enter_context(tc.tile_pool(name="ps_r", bufs=1, space="PSUM"))
        ps_misc = ctx.enter_context(tc.tile_pool(name="ps_misc", bufs=1, space="PSUM"))

        wsb = persist.tile(list(wts_np.shape), F16)
        nc.gpsimd.dma_start(wsb[:], dW[:])

        acc_slots = persist.tile([128, 64], F32)
        ones_f32 = persist.tile([128, 1], F32)

        oi = _WIDX[("ones",)]
        nc.vector.tensor_copy(ones_f32[:], wsb[:, oi * 128:oi * 128 + 1])

        # Sync-clock absorbers. Each 64B ISA instruction fits ~2 sync
        # commands (1 update + 1 wait), so every real op may carry at most
        # ONE cross-proc wait. Tiny engine ops ("carriers") pre-advance each
        # engine's observed clock of one other proc; emission order = sched
        # priority keeps them ahead of the real ops.
        dve_scr = persist.tile([1, 1], F32)
        act_scr = persist.tile([1, 1], F32)
        pool_scr = persist.tile([1, 2], F16)
        pe_scr = ps_misc.tile([1, 64], F32, tag="misc")

        def dve_sees(ap):
            nc.vector.tensor_copy(dve_scr[:], ap[0:1, 0:1])

        def act_sees(ap):
            nc.scalar.copy(act_scr[:], ap[0:1, 0:1])

        def pe_sees(ap):
            nc.tensor.matmul(pe_scr[0:1, 0:1], ap[:, 0:1], ap[:, 0:1],
                             start=True, stop=True)

        # pre-initialize the xu pool slots so partition 0 (never DMA'd) is
        # finite data, not virgin SBUF
        for _ in range(4):
            xu0 = xp.tile([128, 1026], F16, tag="xu")
            nc.vector.memset(xu0[0:1, :], 0.0)
        for _ in range(2):
            for tg in ("a_h", "a_1", "a_2"):
                f0 = fields.tile([128, 1026], F16, tag=tg, name=tg)
                nc.vector.memset(f0[0:1, 1024:1026], 0.0)

        def WT(t, name):
            i = _WIDX[(t, name)]
            return wsb[:, i * 128:(i + 1) * 128]


        def image_pipeline(dram, s, t, off, tag):
            x = xp.tile([128, 1026], F16, tag="x")
            nc.gpsimd.dma_start(x[:, :], dram[s, off:off + 128, :])
            # xu[p] = image row off+p-1 (partition-shifted SBUF copy)
            xu = xp.tile([128, 1026], F16, tag="xu")
            nc.sync.dma_start(xu[1:128, :], x[0:127, :])

            # |diff| fields (fused sub+abs custom DVE op, or native fallback)
            # col conventions (img col of sb col j):
            #   a_v: j-1   a_h: j    a_1: j    a_2: j-1
            # a_v on PE (bidiagonal matrix) + ACT abs; PSUM-relieving the DVE
            dv_ps = ps_dv.tile([128, 1024], F32, tag="dv")
            for c in range(2):
                nc.tensor.matmul(
                    dv_ps[:, c * 512:(c + 1) * 512], WT(t, "DV"),
                    x[:, 1 + c * 512: 513 + c * 512], start=True, stop=True)
            a_v = fields.tile([128, 1024], F16, tag="a_v")
            nc.scalar.activation(a_v[:], dv_ps[:], AF.Abs)

            def absfield(tag_, i0, i1):
                # sub on the 1025 valid cols (odd shift forces 1x mode);
                # abs full 1026-wide so it runs in the 2x packed mode
                f = fields.tile([128, 1026], F16, tag=tag_, name=tag_)
                nc.vector.tensor_sub(f[:, 0:1025], i0, i1)
                if tag_ in ("a_1", "a_2"):
                    # balance: diag-field |.| runs on ScalarE, a_h stays DVE
                    nc.scalar.activation(f[:, 0:1025], f[:, 0:1025], AF.Abs)
                else:
                    nc.vector.scalar_tensor_tensor(
                        f[:, :], f[:, :], -1.0, f[:, :], OP.mult, OP.max)
                return f
            a_h = absfield("a_h", x[:, 1:1026], x[:, 0:1025])
            a_1 = absfield("a_1", x[:, 1:1026], xu[:, 0:1025])
            a_2 = absfield("a_2", x[:, 0:1025], xu[:, 1:1026])


            # sml assembly on PE (PSUM accumulate), 2 chunks of 512 cols
            sml = ps_sml.tile([128, 1024], F32, tag="sml")
            for c in range(2):
                F0 = c * 512
                o = sml[:, F0:F0 + 512]
                mm = nc.tensor.matmul
                mm(o, WT(t, "AV"), a_v[:, F0:F0 + 512], start=True, stop=False)
                mm(o, WT(t, "IHF"), a_h[:, F0:F0 + 512], start=False, stop=False)
                mm(o, WT(t, "IHF1"), a_h[:, F0 + 1:F0 + 513], start=False, stop=False)
                mm(o, WT(t, "I71"), a_1[:, F0:F0 + 512], start=False, stop=False)
                mm(o, WT(t, "S71"), a_1[:, F0 + 1:F0 + 513], start=False, stop=False)
                mm(o, WT(t, "I72"), a_2[:, F0 + 1:F0 + 513], start=False, stop=False)
                mm(o, WT(t, "S72"), a_2[:, F0:F0 + 512], start=False, stop=True)

            s2 = fields.tile([128, 1024], F16, tag=f"s2{tag}")
            nc.scalar.activation(s2[:], sml[:], AF.Square)
            return s2

        for s in range(NSITE):
            for t in range(NT):
                r0, R, off = _tile_geom(t)
                s2a = image_pipeline(dA, s, t, off, "A")
                s2b = image_pipeline(dB, s, t, off, "B")

                q = tails.tile([128, 1026], F16, tag="q")
                nc.vector.memset(q[:, 0:1], 0.0)
                nc.vector.memset(q[:, 1025:1026], 0.0)
                nc.vector.scalar_tensor_tensor(
                    q[:, 1:1025], s2b[:], -1.0, s2a[:], OP.mult, OP.add)

                tt = tails.tile([128, 1024], F16, tag="tt")
                nc.vector.tensor_add(tt[:], q[:, 0:1024], q[:, 2:1026])
                th = tails.tile([128, 1024], F16, tag="th")
                nc.vector.scalar_tensor_tensor(
                    th[:], q[:, 1:1025], 2.0, tt[:], OP.mult, OP.add)

                r = ps_r.tile([128, 1024], F32, tag="r")
                for c in range(2):
                    nc.tensor.matmul(
                        r[:, c * 512:(c + 1) * 512], WT(t, "TRI"),
                        th[:, c * 512:(c + 1) * 512], start=True, stop=True,
                    )

                junk = tails.tile([128, 1024], F32, tag="junk")
                idx = s * NT + t
                nc.scalar.activation(junk[:], r[:], AF.Square,
                                     accum_out=acc_slots[:, idx:idx + 1])

        tot_ps = ps_misc.tile([1, 64], F32, tag="misc2")
        nc.tensor.matmul(tot_ps[:], ones_f32[:], acc_slots[:, 0:64],
                         start=True, stop=True)
        out_sb = persist.tile([1, 1], F32)
        nc.vector.reduce_sum(out_sb[:], tot_ps[:], axis=mybir.AxisListType.X)
        nc.sync.dma_start(dO[:], out_sb[:])

    nc.compile()
    return nc


_WIDX = None


def _get_module():
    global _WIDX
    if "nc" in _CACHE:
        return _CACHE["nc"], _CACHE["wts"]
    wts_np, widx = _build_weights()
    _WIDX = widx
    nc = _build(wts_np)
    _CACHE["nc"] = nc
    _CACHE["wts"] = wts_np
    return nc, wts_np


def _pad_cols(a):
    # [NSITE, H, W] -> [NSITE, H, W+2] with edge-replicated columns
    return np.ascontiguousarray(
        np.concatenate([a[:, :, :1], a, a[:, :, -1:]], axis=2))


def kernel(TensorA, TensorB):
    from concourse.bass_utils import run_bass_kernel_spmd

    nc, wts_np = _get_module()
    A = np.asarray(TensorA, dtype=np.float32).reshape(B * C, H, W)
    Bv = np.asarray(TensorB, dtype=np.float32).reshape(B * C, H, W)
    in_maps = []
    for c in range(NCORES):
        in_maps.append({
            "TA": _pad_cols(A[c * NSITE:(c + 1) * NSITE]),
            "TB": _pad_cols(Bv[c * NSITE:(c + 1) * NSITE]),
            "WTS": wts_np,
        })
    res = run_bass_kernel_spmd(
        nc, in_maps, core_ids=list(range(NCORES)),
        trace=bool(int(os.environ.get("CLAR_TRACE", "0"))),
    )
    _CACHE["last_results"] = res
    total = sum(float(r["OUT"][0, 0]) for r in res.results)
    return np.float32(total * FINAL_SCALE)

